# revision 1
# baseline (speedup 1.0000x reference)
"""Multi-head attention forward on 8 Trainium2 NeuronCores (Bass/Tile).

Problem: B=4, L=2048, D=1024, H=16 heads, DV=64.
  out = softmax((x_q Wq^T + bq)(x_k Wk^T + bk)^T / sqrt(DV)) (x_v Wv^T + bv) Wc^T + bc

Sharding (8 cores): core c handles batch b = c//2 and head-group g = c%2
(8 heads = 512 of the 1024 projection columns). Each core produces a
full-shape [L, D] partial of the output projection (contraction over its
512 attention-output dims); the host sums the two partials per batch and
adds bc.

Per-core pipeline (all matmuls fp32r = full-rate fp32 on the PE):
  A. V projection -> natural layout [2048, 8*65] with a ones column per
     head (rowsum trick), then Q/K projections for head-pair 0
     (QT/KT [128, 2048] per pair).
  B. Per head-pair m: attention for its 2 heads. Inner loop per
     (head, q-half): 16 k-tiles; scores^T [k=128, q=1024] in PSUM ->
     ACT exp (scale=1/8, fused) -> SBUF fp32r; AV accumulates [65, 1024]
     in PSUM (ones row = softmax denominator). ACT exp overlaps the PE stream; AV lags 2 k-tiles so all
     semaphore waits are pre-satisfied and the PE stays dense and
     HAM-warm.  Softmax denominator is applied post-AV on
     [64, 1024] tiles (reciprocal + DRAM-roundtrip partition broadcast).
  C. Output projection tail: out[l, n] accumulated over the 4 d-tiles.
"""

from contextlib import ExitStack

import numpy as np

import concourse.bacc as bacc
import concourse.mybir as mybir
from concourse.tile import TileContext
from concourse.bass_utils import run_bass_kernel_spmd

B, L, D, H = 4, 2048, 1024, 16
DV = 64
HPC = 8           # heads per core
OC = HPC * DV     # 512 projection cols per core
NCORES = 8

F32 = mybir.dt.float32
F32R = mybir.dt.float32r
EXP = mybir.ActivationFunctionType.Exp

NI = D // 128    # 8 contraction tiles for projections
NM = OC // 128   # 4 head pairs
NLT = L // 128   # 16 l/k tiles
QW = 1024        # q-half width in stage B

_CACHE = {}


def _build():
    nc = bacc.Bacc("TRN2", target_bir_lowering=False, debug=False,
                   num_devices=NCORES)

    xtq = nc.dram_tensor("XTQ", [D, L], F32R, kind="ExternalInput")
    xtk = nc.dram_tensor("XTK", [D, L], F32R, kind="ExternalInput")
    xtv = nc.dram_tensor("XTV", [D, L], F32R, kind="ExternalInput")
    wqt = nc.dram_tensor("WQT", [D, OC], F32R, kind="ExternalInput")
    wkt = nc.dram_tensor("WKT", [D, OC], F32R, kind="ExternalInput")
    wvt = nc.dram_tensor("WVT", [D, OC], F32R, kind="ExternalInput")
    wct = nc.dram_tensor("WCT", [OC, D], F32R, kind="ExternalInput")
    bqd = nc.dram_tensor("BQ", [OC], F32, kind="ExternalInput")
    bkd = nc.dram_tensor("BK", [OC], F32, kind="ExternalInput")
    bvd = nc.dram_tensor("BV", [OC], F32, kind="ExternalInput")
    out = nc.dram_tensor("OUT", [L, D], F32, kind="ExternalOutput")

    with TileContext(nc) as tc:
        with (
            tc.tile_pool(name="qkt", bufs=2 * NM) as qkt_pool,
            tc.tile_pool(name="vext", bufs=NLT) as vext_pool,
            tc.tile_pool(name="rcd", bufs=2, space="DRAM") as rcd_pool,
            tc.tile_pool(name="st", bufs=3, space="PSUM") as st_pool,
            tc.tile_pool(name="ot", bufs=1, space="PSUM") as ot_pool,
        ):
            qt = [qkt_pool.tile([128, L], F32R, tag="qkt", name=f"qt{i}")
                  for i in range(NM)]
            kt = [qkt_pool.tile([128, L], F32R, tag="qkt", name=f"kt{i}")
                  for i in range(NM)]
            vext = [vext_pool.tile([128, HPC, DV + 1], F32R, name=f"vext{i}",
                                   tag="vext")
                    for i in range(NLT)]

            # --- stage-A pools (weights, biases, x chunks) ---
            astack = ExitStack()
            xt_pool = astack.enter_context(tc.tile_pool(name="xt", bufs=16))
            w_pool = astack.enter_context(tc.tile_pool(name="w", bufs=NI))
            bias_pool = astack.enter_context(
                tc.tile_pool(name="bias", bufs=2 * NM))
            qb_tile = bias_pool.tile([128, NM], F32, tag="b1", name="bqt",
                                     bufs=2)
            nc.sync.dma_start(
                out=qb_tile, in_=bqd[:].rearrange("(m p) -> p m", p=128))
            kb_tile = bias_pool.tile([128, NM], F32, tag="b1", name="bkt",
                                     bufs=2)
            nc.sync.dma_start(
                out=kb_tile, in_=bkd[:].rearrange("(m p) -> p m", p=128))
            qbias = [qb_tile[:, mm_:mm_ + 1] for mm_ in range(NM)]
            kbias = [kb_tile[:, mm_:mm_ + 1] for mm_ in range(NM)]
            vbias = bias_pool.tile([128, OC], F32, tag="bv", bufs=1)
            nc.sync.dma_start(
                out=vbias, in_=bvd[:].unsqueeze(0).to_broadcast((128, OC)))
            onesf = bias_pool.tile([128, HPC], F32, tag="ones", bufs=1)
            nc.vector.memset(onesf, 1.0)

            # ---- stage A1: V projection ----
            with tc.tile_pool(name="wv", bufs=NI) as wv_pool:
                wv_tiles = []
                xv_list = []
                for i in range(NI):
                    t = wv_pool.tile([128, OC], F32R, tag="wv", name="wvt_t")
                    if i == 0:
                        nc.sync.dma_start(out=t[:, 0:256], in_=wvt[0:128, 0:256])
                        nc.sync.dma_start(out=t[:, 256:512],
                                          in_=wvt[0:128, 256:512])
                    else:
                        nc.sync.dma_start(
                            out=t, in_=wvt[i * 128:(i + 1) * 128, :])
                    wv_tiles.append(t)
                    t = xt_pool.tile([128, 512], F32R, tag="xt", name="xv")
                    if i == 0:
                        nc.gpsimd.dma_start(out=t[:, 0:256], in_=xtv[0:128, 0:256])
                        nc.gpsimd.dma_start(out=t[:, 256:512],
                                            in_=xtv[0:128, 256:512])
                    else:
                        nc.gpsimd.dma_start(
                            out=t, in_=xtv[i * 128:(i + 1) * 128, 0:512])
                    xv_list.append(t)
                xv_chunks = {0: xv_list}
                wq_tiles, wk_tiles = [], []
                for lc in range(4):
                    if lc in xv_chunks:
                        x_tiles = xv_chunks[lc]
                    else:
                        x_tiles = []
                        for i in range(NI):
                            t = xt_pool.tile([128, 512], F32R, tag="xt",
                                             name="xv")
                            nc.sync.dma_start(
                                out=t,
                                in_=xtv[i * 128:(i + 1) * 128,
                                        lc * 512:(lc + 1) * 512])
                            x_tiles.append(t)
                    if lc == 1:
                        for i in range(NI):
                            t = w_pool.tile([128, OC], F32R, tag="wq",
                                            name="wqt_t")
                            nc.sync.dma_start(
                                out=t, in_=wqt[i * 128:(i + 1) * 128, :])
                            wq_tiles.append(t)
                    elif lc == 2:
                        for i in range(NI):
                            t = w_pool.tile([128, OC], F32R, tag="wk",
                                            name="wkt_t")
                            nc.sync.dma_start(
                                out=t, in_=wkt[i * 128:(i + 1) * 128, :])
                            wk_tiles.append(t)
                    for lsp in range(2):   # two l-subtiles share one psum tile
                        ps = st_pool.tile([128, QW], F32, tag="st", name="psv")
                        for sub in range(2):
                            ls = lsp * 2 + sub
                            for i in range(NI):
                                nc.tensor.matmul(
                                    ps[:, sub * 512:(sub + 1) * 512],
                                    lhsT=x_tiles[i][:, ls * 128:(ls + 1) * 128],
                                    rhs=wv_tiles[i],
                                    start=(i == 0), stop=(i == NI - 1))
                        for sub in range(2):
                            lt = lc * 4 + lsp * 2 + sub
                            nc.vector.tensor_add(
                                vext[lt][:, :, 0:DV],
                                ps[:, sub * 512:(sub + 1) * 512].rearrange(
                                    "p (h d) -> p h d", h=HPC),
                                vbias.rearrange("p (h d) -> p h d", h=HPC))
                            nc.vector.tensor_copy(vext[lt][:, :, DV], onesf)

            # ---- stage A2: Q/K projections, all pairs, one pass over xT ----
            for w_tiles, xsrc, dst, biases in (
                (wq_tiles, xtq, qt, qbias),
                (wk_tiles, xtk, kt, kbias),
            ):
                for lc in range(4):
                    x_tiles = []
                    for i in range(NI):
                        t = xt_pool.tile([128, 512], F32R, tag="xt", name="xp")
                        nc.sync.dma_start(
                            out=t,
                            in_=xsrc[i * 128:(i + 1) * 128,
                                     lc * 512:(lc + 1) * 512])
                        x_tiles.append(t)
                    for mp in range(2):
                        ps = st_pool.tile([128, QW], F32, tag="st", name="psp")
                        for sub in range(2):
                            m = mp * 2 + sub
                            for i in range(NI):
                                nc.tensor.matmul(
                                    ps[:, sub * 512:(sub + 1) * 512],
                                    lhsT=w_tiles[i][:, m * 128:(m + 1) * 128],
                                    rhs=x_tiles[i],
                                    start=(i == 0), stop=(i == NI - 1))
                        for sub in range(2):
                            m = mp * 2 + sub
                            nc.vector.tensor_add(
                                dst[m][:, lc * 512:(lc + 1) * 512],
                                ps[:, sub * 512:(sub + 1) * 512],
                                biases[m].to_broadcast((128, 512)))

            astack.close()

            # ---- stage B: attention per pair ----
            attnt = {}
            bstack = ExitStack()
            attnt_pool = bstack.enter_context(tc.tile_pool(name="attnt",
                                                           bufs=NM))
            ex_pool = bstack.enter_context(tc.tile_pool(name="ex", bufs=3))
            oc_pool = bstack.enter_context(tc.tile_pool(name="oc", bufs=2))
            rr_pool = bstack.enter_context(tc.tile_pool(name="rr", bufs=1))
            wc_pool = bstack.enter_context(tc.tile_pool(name="wc", bufs=NM))
            ob_pool = bstack.enter_context(tc.tile_pool(name="ob", bufs=3))
            wc_tiles = []
            for dt in range(NM):
                t = wc_pool.tile([128, D], F32R, tag="wc", name="wct_t")
                nc.sync.dma_start(out=t, in_=wct[dt * 128:(dt + 1) * 128, :])
                wc_tiles.append(t)
            for m in range(NM):
                attnt[m] = attnt_pool.tile([128, L], F32R, tag="attnt",
                                           name=f"attnt{m}")
                for h2 in range(2):
                    h = m * 2 + h2
                    off = h2 * DV
                    for qc in range(2):
                        ot = ot_pool.tile([DV + 1, QW], F32, tag="ot",
                                          name="ot")
                        sts = {}
                        exs = {}

                        def st_step(k):
                            st = st_pool.tile([128, QW], F32, tag="st",
                                              name="st")
                            for j in range(2):
                                nc.tensor.matmul(
                                    st[:, j * 512:(j + 1) * 512],
                                    lhsT=kt[m][off:off + DV,
                                               k * 128:(k + 1) * 128],
                                    rhs=qt[m][off:off + DV,
                                              qc * QW + j * 512:
                                              qc * QW + (j + 1) * 512],
                                    start=True, stop=True)
                            ex = ex_pool.tile([128, QW], F32R, tag="ex",
                                              name="ex")
                            nc.scalar.activation(
                                out=ex, in_=st, func=EXP, scale=0.125)
                            exs[k] = ex

                        def av_step(k):
                            ex = exs.pop(k)
                            for j in range(2):
                                nc.tensor.matmul(
                                    ot[:, j * 512:(j + 1) * 512],
                                    lhsT=vext[k][:, h, :],
                                    rhs=ex[:, j * 512:(j + 1) * 512],
                                    start=(k == 0), stop=(k == NLT - 1))

                        for k in range(NLT):
                            st_step(k)
                            if k >= 2:
                                av_step(k - 2)
                        av_step(NLT - 2)
                        av_step(NLT - 1)

                        # normalization: stage ot out of PSUM, then scale
                        ocs = oc_pool.tile([DV + 1, QW], F32, tag="oc",
                                           name="ocs")
                        nc.vector.tensor_copy(ocs, ot)
                        rc = rr_pool.tile([1, QW], F32, tag="rc", name="rc")
                        nc.vector.reciprocal(rc, ocs[DV:DV + 1, :])
                        rcd = rcd_pool.tile([QW], F32, tag="rcd", name="rcd")
                        nc.sync.dma_start(out=rcd[:].unsqueeze(0), in_=rc)
                        rb = rr_pool.tile([DV, QW], F32, tag="rb", name="rb")
                        nc.gpsimd.dma_start(
                            out=rb,
                            in_=rcd[:].unsqueeze(0).to_broadcast((DV, QW)))
                        nc.vector.tensor_mul(
                            attnt[m][off:off + DV, qc * QW:(qc + 1) * QW],
                            ocs[0:DV, :],
                            rb)

            # ---- stage C: output projection ----
            for lt in range(NLT):
                ps = st_pool.tile([128, QW], F32, tag="st", name="pso")
                for nck in range(2):
                    for dt in range(NM):
                        nc.tensor.matmul(
                            ps[:, nck * 512:(nck + 1) * 512],
                            lhsT=attnt[dt][:, lt * 128:(lt + 1) * 128],
                            rhs=wc_tiles[dt][:, nck * 512:(nck + 1) * 512],
                            start=(dt == 0), stop=(dt == NM - 1))
                ob = ob_pool.tile([128, QW], F32, tag="ob", name="ob")
                nc.vector.tensor_copy(ob, ps)
                nc.sync.dma_start(
                    out=out[lt * 128:(lt + 1) * 128, :], in_=ob)
            bstack.close()

    nc.compile()
    return nc


def _get_nc():
    if "nc" not in _CACHE:
        _CACHE["nc"] = _build()
    return _CACHE["nc"]


def kernel(query, key, value, Wq, bq, Wk, bk, Wv, bv, Wc, bc, **_unused):
    query = np.asarray(query, np.float32)
    key = np.asarray(key, np.float32)
    value = np.asarray(value, np.float32)
    Wq = np.asarray(Wq, np.float32)
    Wk = np.asarray(Wk, np.float32)
    Wv = np.asarray(Wv, np.float32)
    Wc = np.asarray(Wc, np.float32)
    bq = np.asarray(bq, np.float32)
    bk = np.asarray(bk, np.float32)
    bv = np.asarray(bv, np.float32)
    bc = np.asarray(bc, np.float32)

    nc = _get_nc()

    xtq = [np.ascontiguousarray(query[b].T) for b in range(B)]
    xtk = [np.ascontiguousarray(key[b].T) for b in range(B)]
    xtv = [np.ascontiguousarray(value[b].T) for b in range(B)]
    wqt_g = [np.ascontiguousarray(Wq[g * OC:(g + 1) * OC, :].T) for g in range(2)]
    wkt_g = [np.ascontiguousarray(Wk[g * OC:(g + 1) * OC, :].T) for g in range(2)]
    wvt_g = [np.ascontiguousarray(Wv[g * OC:(g + 1) * OC, :].T) for g in range(2)]
    wct_g = [np.ascontiguousarray(Wc[:, g * OC:(g + 1) * OC].T) for g in range(2)]

    in_maps = []
    for c in range(NCORES):
        b, g = c // 2, c % 2
        in_maps.append({
            "XTQ": xtq[b], "XTK": xtk[b], "XTV": xtv[b],
            "WQT": wqt_g[g], "WKT": wkt_g[g], "WVT": wvt_g[g],
            "WCT": wct_g[g],
            "BQ": np.ascontiguousarray(bq[g * OC:(g + 1) * OC]),
            "BK": np.ascontiguousarray(bk[g * OC:(g + 1) * OC]),
            "BV": np.ascontiguousarray(bv[g * OC:(g + 1) * OC]),
        })

    res = run_bass_kernel_spmd(nc, in_maps, core_ids=list(range(NCORES)),
                               **_CACHE.get("run_kwargs", {}))
    _CACHE["last_results"] = res

    outp = np.empty((B, L, D), np.float32)
    for b in range(B):
        outp[b] = res.results[2 * b]["OUT"] + res.results[2 * b + 1]["OUT"]
    outp += bc
    return outp



# revision 5
# speedup vs baseline: 1.0597x; 1.0597x over previous
"""Multi-head attention forward on 8 Trainium2 NeuronCores (Bass/Tile).

Problem: B=4, L=2048, D=1024, H=16 heads, DV=64.
  out = softmax((x_q Wq^T)(x_k Wk^T)^T / sqrt(DV)) (x_v Wv^T) Wc^T + biases

Sharding (8 cores): core c handles batch b = c//2 and head-group g = c%2
(8 heads = 512 of the 1024 projection columns). Each core produces a
full-shape [L, D] partial of the output projection; the host sums the two
partials per batch and adds bc.

All device data is bf16 (PSUM accumulation fp32); inputs are converted on
the host. Per-core pipeline:
  A1. V projection -> vext [128, 8, 65] bf16 per l-tile (ones column per
      head gives the softmax denominator through the AV matmul).
  A2. Q/K projections per head-pair -> qt/kt [128(2x64 dims), 2048] bf16.
  B.  Per (head, q-half) unit: 16 k-tiles of scores^T [k=128, q=1024] in
      PSUM -> ACT exp (scale=1/8) -> ex bf16; then AV flipped:
      out[q=128, 65] chains (lhsT=ex slice, rhs=vext) -> per-partition
      denominator -> reciprocal_approx_fast + tensor_scalar normalize
      into att_s [128, 64, 16]. Per head, one SBUF->SBUF DMA xbar
      transpose produces attnt [64, 2048] (= attn^T) for stage C.
      A1 matmuls are interleaved into B(0) score slots and A2(m+1)
      matmuls into B(m) slots so the PE fills the gaps of the ACT-bound
      exp stream.
  C.  Output projection out[l, 1024] accumulated over the 4 head pairs.
"""

import numpy as np

import concourse.bacc as bacc
import concourse.mybir as mybir
from concourse.tile import TileContext
from concourse.bass_utils import run_bass_kernel_spmd

try:
    from ml_dtypes import bfloat16 as np_bf16
except ImportError:  # pragma: no cover
    import jax.numpy as jnp
    np_bf16 = jnp.bfloat16

B, L, D, H = 4, 2048, 1024, 16
DV = 64
HPC = 8           # heads per core
OC = HPC * DV     # 512 projection cols per core
NCORES = 8

F32 = mybir.dt.float32
BF16 = mybir.dt.bfloat16
EXP = mybir.ActivationFunctionType.Exp
MULT = mybir.AluOpType.mult
ADD = mybir.AluOpType.add

NI = D // 128    # 8 contraction tiles for projections
NM = OC // 128   # 4 head pairs
NLT = L // 128   # 16 l/k tiles
QW = 1024        # q-half width in stage B

_CACHE = {}
PIPE = True
INTERLEAVE = True


def _build():
    nc = bacc.Bacc("TRN2", target_bir_lowering=False, debug=False,
                   num_devices=NCORES)

    xtq = nc.dram_tensor("XTQ", [D, L], BF16, kind="ExternalInput")
    xtk = nc.dram_tensor("XTK", [D, L], BF16, kind="ExternalInput")
    xtv = nc.dram_tensor("XTV", [D, L], BF16, kind="ExternalInput")
    wqt = nc.dram_tensor("WQT", [D, OC], BF16, kind="ExternalInput")
    wkt = nc.dram_tensor("WKT", [D, OC], BF16, kind="ExternalInput")
    wvt = nc.dram_tensor("WVT", [D, OC], BF16, kind="ExternalInput")
    wct = nc.dram_tensor("WCT", [OC, D], BF16, kind="ExternalInput")
    bqd = nc.dram_tensor("BQ", [OC], F32, kind="ExternalInput")
    bkd = nc.dram_tensor("BK", [OC], F32, kind="ExternalInput")
    bvd = nc.dram_tensor("BV", [OC], F32, kind="ExternalInput")
    out = nc.dram_tensor("OUT", [L, D], BF16, kind="ExternalOutput")

    with TileContext(nc) as tc:
        with (
            tc.tile_pool(name="const", bufs=1) as const_pool,
            tc.tile_pool(name="wcp", bufs=NM) as wc_pool,
            tc.tile_pool(name="wvp", bufs=NI) as wv_pool,
            tc.tile_pool(name="wqp", bufs=NI) as wq_pool,
            tc.tile_pool(name="wkp", bufs=NI) as wk_pool,
            tc.tile_pool(name="xt", bufs=16) as xt_pool,
            tc.tile_pool(name="qkt", bufs=2 * NM) as qkt_pool,
            tc.tile_pool(name="vext", bufs=NLT) as vext_pool,
            tc.tile_pool(name="ex", bufs=32) as ex_pool,
            tc.tile_pool(name="atts", bufs=2) as atts_pool,
            tc.tile_pool(name="attnt", bufs=NM) as attnt_pool,
            tc.tile_pool(name="rcp", bufs=4) as rcp_pool,
            tc.tile_pool(name="ob", bufs=3) as ob_pool,
            tc.tile_pool(name="st", bufs=2, space="PSUM") as st_pool,
            tc.tile_pool(name="ap", bufs=1, space="PSUM") as ap_pool,
            tc.tile_pool(name="av", bufs=2, space="PSUM") as av_pool,
        ):
            # ---- constants / weights ----
            qb_tile = const_pool.tile([128, NM], F32, tag="bq", name="bqt")
            nc.sync.dma_start(
                out=qb_tile, in_=bqd[:].rearrange("(m p) -> p m", p=128))
            kb_tile = const_pool.tile([128, NM], F32, tag="bk", name="bkt")
            nc.sync.dma_start(
                out=kb_tile, in_=bkd[:].rearrange("(m p) -> p m", p=128))
            vbias = const_pool.tile([128, OC], F32, tag="bv", name="bvt")
            nc.sync.dma_start(
                out=vbias, in_=bvd[:].unsqueeze(0).to_broadcast((128, OC)))
            onesf = const_pool.tile([128, HPC], F32, tag="ones", name="ones")
            nc.vector.memset(onesf, 1.0)

            wc_tiles = []
            for dt in range(NM):
                t = wc_pool.tile([128, D], BF16, tag="wc", name="wct_t")
                nc.sync.dma_start(out=t, in_=wct[dt * 128:(dt + 1) * 128, :])
                wc_tiles.append(t)
            wv_tiles = []
            for i in range(NI):
                t = wv_pool.tile([128, OC], BF16, tag="wv", name="wvt_t")
                nc.sync.dma_start(out=t, in_=wvt[i * 128:(i + 1) * 128, :])
                wv_tiles.append(t)
            wq_tiles = []
            for i in range(NI):
                t = wq_pool.tile([128, OC], BF16, tag="wq", name="wqt_t")
                nc.sync.dma_start(out=t, in_=wqt[i * 128:(i + 1) * 128, :])
                wq_tiles.append(t)
            wk_tiles = []
            for i in range(NI):
                t = wk_pool.tile([128, OC], BF16, tag="wk", name="wkt_t")
                nc.sync.dma_start(out=t, in_=wkt[i * 128:(i + 1) * 128, :])
                wk_tiles.append(t)

            qt = [qkt_pool.tile([128, L], BF16, tag="qkt", name=f"qt{i}")
                  for i in range(NM)]
            kt = [qkt_pool.tile([128, L], BF16, tag="qkt", name=f"kt{i}")
                  for i in range(NM)]
            vext = [vext_pool.tile([128, HPC, DV + 1], BF16, tag="vext",
                                   name=f"vext{i}")
                    for i in range(NLT)]

            # ---- stage A1 generator: V projection (128 mms + DVE) ----
            # Yields BEFORE each matmul (except the first) so the pull that
            # consumes the last matmul also emits the trailing DVE writes.
            def a1_gen():
                first = [True]

                def tick():
                    if first[0]:
                        first[0] = False
                        return iter(())
                    return iter((None,))

                for lc in range(4):
                    x_tiles = []
                    for i in range(NI):
                        t = xt_pool.tile([128, 512], BF16, tag="xt", name="xv")
                        nc.sync.dma_start(
                            out=t,
                            in_=xtv[i * 128:(i + 1) * 128,
                                    lc * 512:(lc + 1) * 512])
                        x_tiles.append(t)
                    for lsp in range(2):
                        ps = ap_pool.tile([128, QW], F32, tag="ap", name="psv")
                        for sub in range(2):
                            ls = lsp * 2 + sub
                            for i in range(NI):
                                yield from tick()
                                nc.tensor.matmul(
                                    ps[:, sub * 512:(sub + 1) * 512],
                                    lhsT=x_tiles[i][:, ls * 128:(ls + 1) * 128],
                                    rhs=wv_tiles[i],
                                    start=(i == 0), stop=(i == NI - 1))
                        for sub in range(2):
                            lt = lc * 4 + lsp * 2 + sub
                            nc.vector.tensor_add(
                                vext[lt][:, :, 0:DV],
                                ps[:, sub * 512:(sub + 1) * 512].rearrange(
                                    "p (h d) -> p h d", h=HPC),
                                vbias.rearrange("p (h d) -> p h d", h=HPC))
                            nc.vector.tensor_copy(vext[lt][:, :, DV], onesf)

            # ---- stage A2 generator: Q/K projection for pair m (64 mms) ----
            def a2_gen(m):
                first = [True]

                def tick():
                    if first[0]:
                        first[0] = False
                        return iter(())
                    return iter((None,))

                for w_tiles, xsrc, dst, btile in (
                    (wq_tiles, xtq, qt[m], qb_tile),
                    (wk_tiles, xtk, kt[m], kb_tile),
                ):
                    for lcp in range(2):
                        x_tiles = {}
                        for sub in range(2):
                            lc = lcp * 2 + sub
                            xl = []
                            for i in range(NI):
                                t = xt_pool.tile([128, 512], BF16, tag="xt",
                                                 name="xp")
                                nc.sync.dma_start(
                                    out=t,
                                    in_=xsrc[i * 128:(i + 1) * 128,
                                             lc * 512:(lc + 1) * 512])
                                xl.append(t)
                            x_tiles[sub] = xl
                        ps = ap_pool.tile([128, QW], F32, tag="ap", name="psp")
                        for sub in range(2):
                            for i in range(NI):
                                yield from tick()
                                nc.tensor.matmul(
                                    ps[:, sub * 512:(sub + 1) * 512],
                                    lhsT=w_tiles[i][:, m * 128:(m + 1) * 128],
                                    rhs=x_tiles[sub][i],
                                    start=(i == 0), stop=(i == NI - 1))
                        for sub in range(2):
                            lc = lcp * 2 + sub
                            nc.vector.tensor_scalar(
                                out=dst[:, lc * 512:(lc + 1) * 512],
                                in0=ps[:, sub * 512:(sub + 1) * 512],
                                scalar1=btile[:, m:m + 1],
                                scalar2=None, op0=ADD)

            def drain(gen):
                if gen is None:
                    return
                for _ in gen:
                    pass

            # ---- pre-B: A2(0) ----
            drain(a2_gen(0))

            # ---- stage B ----
            # Units: (m, h2, qc); AV of unit u runs after scores of u+1.
            # Interleave: A1 into B(0) units 0-1 (4 mms/slot), A2(m+1) into
            # B(m) units 2-3 (2 mms/slot).
            units = []
            for m in range(NM):
                for h2 in range(2):
                    for qc in range(2):
                        units.append((m, h2, qc))

            exs = {}        # unit idx -> list of 16 ex tiles
            att_s = {}      # head -> staging tile
            pend_av = []    # queue of units whose AV is not yet emitted

            def emit_scores(ui, intl_gen, n_intl):
                m, h2, qc = units[ui]
                off = h2 * DV
                if m not in att_s:
                    att_s[m] = atts_pool.tile(
                        [128, NLT, 128], BF16, tag="atts", name=f"atts{m}")
                ex_list = []
                for k in range(NLT):
                    if intl_gen is not None:
                        for _ in range(n_intl):
                            if next(intl_gen, "END") == "END":
                                intl_gen = None
                                break
                    st = st_pool.tile([128, QW], F32, tag="st", name="st")
                    for j in range(2):
                        nc.tensor.matmul(
                            st[:, j * 512:(j + 1) * 512],
                            lhsT=kt[m][off:off + DV, k * 128:(k + 1) * 128],
                            rhs=qt[m][off:off + DV,
                                      qc * QW + j * 512:qc * QW + (j + 1) * 512],
                            start=True, stop=True)
                    ex = ex_pool.tile([128, QW], BF16, tag="ex", name="ex")
                    nc.scalar.activation(out=ex, in_=st, func=EXP, scale=0.125)
                    ex_list.append(ex)
                exs[ui] = ex_list
                return intl_gen

            def emit_av(ui):
                m, h2, qc = units[ui]
                h = m * 2 + h2
                ex_list = exs.pop(ui)
                ast = att_s[m]
                for qsg in range(2):
                    av = av_pool.tile([128, 4, DV + 1], F32, tag="av",
                                      name="av")
                    for qs in range(4):
                        qq = (qsg * 4 + qs) * 128
                        for k in range(NLT):
                            nc.tensor.matmul(
                                av[:, qs, :],
                                lhsT=ex_list[k][:, qq:qq + 128],
                                rhs=vext[k][:, h, :],
                                start=(k == 0), stop=(k == NLT - 1))
                    rc = rcp_pool.tile([128, 4], F32, tag="rc", name="rc")
                    nc.vector.reciprocal_approx_fast(out=rc, in_=av[:, :, DV])
                    for qs in range(4):
                        lt = qc * 8 + qsg * 4 + qs
                        nc.vector.tensor_scalar(
                            out=ast[:, lt, h2 * DV:(h2 + 1) * DV],
                            in0=av[:, qs, 0:DV],
                            scalar1=rc[:, qs:qs + 1],
                            scalar2=None, op0=MULT)
                if h2 == 1 and qc == 1:
                    # whole pair staged: xbar semantics
                    # out[fm, b, q] = in[q, b*128 + fm], fm = h2*64+dv
                    if m not in attnt:
                        attnt[m] = attnt_pool.tile([128, L], BF16, tag="attnt",
                                                   name=f"attnt{m}")
                    dst = attnt[m][:, :].rearrange("p (t q) -> p t q", t=NLT)
                    nc.sync.dma_start_transpose(dst, ast)
                    del att_s[m]

            attnt = {}
            if INTERLEAVE:
                intl = a1_gen()
            else:
                drain(a1_gen())
                for _m in range(1, NM):
                    drain(a2_gen(_m))
                intl = None
            n_per_slot = 4
            depth = 1 if PIPE else 0
            for ui in range(len(units)):
                m, h2, qc = units[ui]
                # refresh interleave source at unit boundaries
                if INTERLEAVE and ui % 4 == 2 and m < NM - 1:
                    drain(intl)
                    intl = a2_gen(m + 1)
                    n_per_slot = 2
                intl = emit_scores(ui, intl, n_per_slot)
                pend_av.append(ui)
                if len(pend_av) > depth:
                    emit_av(pend_av.pop(0))
            drain(intl)
            while pend_av:
                emit_av(pend_av.pop(0))

            # ---- stage C: output projection ----
            for lt in range(NLT):
                ps = st_pool.tile([128, QW], F32, tag="st", name="pso")
                for nck in range(2):
                    for dt in range(NM):
                        nc.tensor.matmul(
                            ps[:, nck * 512:(nck + 1) * 512],
                            lhsT=attnt[dt][:, lt * 128:(lt + 1) * 128],
                            rhs=wc_tiles[dt][:, nck * 512:(nck + 1) * 512],
                            start=(dt == 0), stop=(dt == NM - 1))
                ob = ob_pool.tile([128, QW], BF16, tag="ob", name="ob")
                if lt % 2 == 0:
                    nc.vector.tensor_copy(ob, ps)
                else:
                    nc.scalar.copy(ob, ps)
                nc.sync.dma_start(out=out[lt * 128:(lt + 1) * 128, :], in_=ob)

    nc.compile()
    return nc


def _get_nc():
    if "nc" not in _CACHE:
        _CACHE["nc"] = _build()
    return _CACHE["nc"]


def kernel(query, key, value, Wq, bq, Wk, bk, Wv, bv, Wc, bc, **_unused):
    query = np.asarray(query, np.float32)
    key = np.asarray(key, np.float32)
    value = np.asarray(value, np.float32)
    Wq = np.asarray(Wq, np.float32)
    Wk = np.asarray(Wk, np.float32)
    Wv = np.asarray(Wv, np.float32)
    Wc = np.asarray(Wc, np.float32)
    bq = np.asarray(bq, np.float32)
    bk = np.asarray(bk, np.float32)
    bv = np.asarray(bv, np.float32)
    bc = np.asarray(bc, np.float32)

    nc = _get_nc()

    def tb(x):
        return np.ascontiguousarray(x).astype(np_bf16)

    xtq = [tb(query[b].T) for b in range(B)]
    xtk = [tb(key[b].T) for b in range(B)]
    xtv = [tb(value[b].T) for b in range(B)]
    wqt_g = [tb(Wq[g * OC:(g + 1) * OC, :].T) for g in range(2)]
    wkt_g = [tb(Wk[g * OC:(g + 1) * OC, :].T) for g in range(2)]
    wvt_g = [tb(Wv[g * OC:(g + 1) * OC, :].T) for g in range(2)]
    wct_g = [tb(Wc[:, g * OC:(g + 1) * OC].T) for g in range(2)]

    in_maps = []
    for c in range(NCORES):
        b, g = c // 2, c % 2
        in_maps.append({
            "XTQ": xtq[b], "XTK": xtk[b], "XTV": xtv[b],
            "WQT": wqt_g[g], "WKT": wkt_g[g], "WVT": wvt_g[g],
            "WCT": wct_g[g],
            "BQ": np.ascontiguousarray(bq[g * OC:(g + 1) * OC]),
            "BK": np.ascontiguousarray(bk[g * OC:(g + 1) * OC]),
            "BV": np.ascontiguousarray(bv[g * OC:(g + 1) * OC]),
        })

    res = run_bass_kernel_spmd(nc, in_maps, core_ids=list(range(NCORES)),
                               **_CACHE.get("run_kwargs", {}))
    _CACHE["last_results"] = res

    outp = np.empty((B, L, D), np.float32)
    for b in range(B):
        outp[b] = (res.results[2 * b]["OUT"].astype(np.float32)
                   + res.results[2 * b + 1]["OUT"].astype(np.float32))
    outp += bc
    return outp


# revision 8
# speedup vs baseline: 1.1283x; 1.0647x over previous
"""Multi-head attention forward on 8 Trainium2 NeuronCores (Bass/Tile).

Problem: B=4, L=2048, D=1024, H=16 heads, DV=64.
  out = softmax((x_q Wq^T)(x_k Wk^T)^T / sqrt(DV)) (x_v Wv^T) Wc^T + biases

Sharding (8 cores): core c handles batch b = c//2 and head-group g = c%2
(8 heads = 512 of the 1024 projection columns). Each core produces a
full-shape [L, D] partial of the output projection; the host sums the two
partials per batch and adds bc.

All device data is bf16 (PSUM accumulation fp32); inputs are converted on
the host. Per-core pipeline:
  A1. V projection -> vext [128, 8, 65] bf16 per l-tile (ones column per
      head gives the softmax denominator through the AV matmul).
  A2. Q/K projections per head-pair -> qt/kt [128(2x64 dims), 2048] bf16.
  B.  Per (head, q-half) unit: 16 k-tiles of scores^T [k=128, q=1024] in
      PSUM -> ACT exp (scale=1/8) -> ex bf16; then AV flipped:
      out[q=128, 65] chains (lhsT=ex slice, rhs=vext) -> per-partition
      denominator -> reciprocal_approx_fast + tensor_scalar normalize
      into att_s [128, 64, 16]. Per head, one SBUF->SBUF DMA xbar
      transpose produces attnt [64, 2048] (= attn^T) for stage C.
      A1 matmuls are interleaved into B(0) score slots and A2(m+1)
      matmuls into B(m) slots so the PE fills the gaps of the ACT-bound
      exp stream.
  C.  Output projection out[l, 1024] accumulated over the 4 head pairs.
"""

import numpy as np

import concourse.bacc as bacc
import concourse.mybir as mybir
from concourse.tile import TileContext
from concourse.bass_utils import run_bass_kernel_spmd

try:
    from ml_dtypes import bfloat16 as np_bf16
except ImportError:  # pragma: no cover
    import jax.numpy as jnp
    np_bf16 = jnp.bfloat16

B, L, D, H = 4, 2048, 1024, 16
DV = 64
HPC = 8           # heads per core
OC = HPC * DV     # 512 projection cols per core
NCORES = 8

F32 = mybir.dt.float32
BF16 = mybir.dt.bfloat16
EXP = mybir.ActivationFunctionType.Exp
MULT = mybir.AluOpType.mult
ADD = mybir.AluOpType.add

NI = D // 128    # 8 contraction tiles for projections
NM = OC // 128   # 4 head pairs
NLT = L // 128   # 16 l/k tiles
QW = 1024        # q-half width in stage B

_CACHE = {}
PIPE = True
INTERLEAVE = True


def _build():
    nc = bacc.Bacc("TRN2", target_bir_lowering=False, debug=False,
                   num_devices=NCORES)

    xtq = nc.dram_tensor("XTQ", [D, L], BF16, kind="ExternalInput")
    xtk = nc.dram_tensor("XTK", [D, L], BF16, kind="ExternalInput")
    xtv = nc.dram_tensor("XTV", [D, L], BF16, kind="ExternalInput")
    wqt = nc.dram_tensor("WQT", [D, OC], BF16, kind="ExternalInput")
    wkt = nc.dram_tensor("WKT", [D, OC], BF16, kind="ExternalInput")
    wvt = nc.dram_tensor("WVT", [D, OC], BF16, kind="ExternalInput")
    wct = nc.dram_tensor("WCT", [OC, D], BF16, kind="ExternalInput")
    bqd = nc.dram_tensor("BQ", [OC], F32, kind="ExternalInput")
    bkd = nc.dram_tensor("BK", [OC], F32, kind="ExternalInput")
    bvd = nc.dram_tensor("BV", [OC], F32, kind="ExternalInput")
    out = nc.dram_tensor("OUT", [L, D], BF16, kind="ExternalOutput")

    with TileContext(nc) as tc:
        with (
            tc.tile_pool(name="const", bufs=1) as const_pool,
            tc.tile_pool(name="wcp", bufs=1) as wc_pool,
            tc.tile_pool(name="wvp", bufs=1) as wv_pool,
            tc.tile_pool(name="wqp", bufs=1) as wq_pool,
            tc.tile_pool(name="wkp", bufs=1) as wk_pool,
            tc.tile_pool(name="xt", bufs=4) as xt_pool,
            tc.tile_pool(name="qkt", bufs=2 * NM) as qkt_pool,
            tc.tile_pool(name="vext", bufs=NLT) as vext_pool,
            tc.tile_pool(name="ex", bufs=32) as ex_pool,
            tc.tile_pool(name="atts", bufs=2) as atts_pool,
            tc.tile_pool(name="attnt", bufs=NM) as attnt_pool,
            tc.tile_pool(name="rcp", bufs=4) as rcp_pool,
            tc.tile_pool(name="ob", bufs=2) as ob_pool,
            tc.tile_pool(name="st", bufs=2, space="PSUM") as st_pool,
            tc.tile_pool(name="ap", bufs=1, space="PSUM") as ap_pool,
            tc.tile_pool(name="av", bufs=2, space="PSUM") as av_pool,
        ):
            # ---- constants / weights ----
            qb_tile = const_pool.tile([128, NM], F32, tag="bq", name="bqt")
            nc.sync.dma_start(
                out=qb_tile, in_=bqd[:].rearrange("(m p) -> p m", p=128))
            kb_tile = const_pool.tile([128, NM], F32, tag="bk", name="bkt")
            nc.sync.dma_start(
                out=kb_tile, in_=bkd[:].rearrange("(m p) -> p m", p=128))
            vbias = const_pool.tile([128, OC], F32, tag="bv", name="bvt")
            nc.sync.dma_start(
                out=vbias, in_=bvd[:].unsqueeze(0).to_broadcast((128, OC)))
            onesf = const_pool.tile([128, HPC], F32, tag="ones", name="ones")
            nc.vector.memset(onesf, 1.0)

            wq_all = wq_pool.tile([128, NI, OC], BF16, tag="wq", name="wq_a")
            nc.sync.dma_start(
                out=wq_all, in_=wqt[:, :].rearrange("(i p) c -> p i c", p=128))
            wk_all = wk_pool.tile([128, NI, OC], BF16, tag="wk", name="wk_a")
            nc.sync.dma_start(
                out=wk_all, in_=wkt[:, :].rearrange("(i p) c -> p i c", p=128))
            wq_tiles = [wq_all[:, i, :] for i in range(NI)]
            wk_tiles = [wk_all[:, i, :] for i in range(NI)]

            wv_all = wv_pool.tile([128, NI, OC], BF16, tag="wv", name="wv_a")
            wv_tiles = [wv_all[:, i, :] for i in range(NI)]
            wc_all = wc_pool.tile([128, NM, D], BF16, tag="wc", name="wc_a")
            wc_tiles = [wc_all[:, i, :] for i in range(NM)]

            qt = [qkt_pool.tile([128, L], BF16, tag="qkt", name=f"qt{i}")
                  for i in range(NM)]
            kt = [qkt_pool.tile([128, L], BF16, tag="qkt", name=f"kt{i}")
                  for i in range(NM)]
            vext = [vext_pool.tile([128, HPC, DV + 1], BF16, tag="vext",
                                   name=f"vext{i}")
                    for i in range(NLT)]

            # ---- stage A1 generator: V projection (128 mms + DVE) ----
            # Yields BEFORE each matmul (except the first) so the pull that
            # consumes the last matmul also emits the trailing DVE writes.
            def a1_gen():
                first = [True]

                def tick():
                    if first[0]:
                        first[0] = False
                        return iter(())
                    return iter((None,))

                for lc in range(4):
                    xa = xt_pool.tile([128, NI, 512], BF16, tag="xt",
                                      name="xv")
                    nc.sync.dma_start(
                        out=xa,
                        in_=xtv[:, lc * 512:(lc + 1) * 512].rearrange(
                            "(i p) c -> p i c", p=128))
                    x_tiles = [xa[:, i, :] for i in range(NI)]
                    for lsp in range(2):
                        ps = ap_pool.tile([128, QW], F32, tag="ap", name="psv")
                        for sub in range(2):
                            ls = lsp * 2 + sub
                            for i in range(NI):
                                yield from tick()
                                nc.tensor.matmul(
                                    ps[:, sub * 512:(sub + 1) * 512],
                                    lhsT=x_tiles[i][:, ls * 128:(ls + 1) * 128],
                                    rhs=wv_tiles[i],
                                    start=(i == 0), stop=(i == NI - 1))
                        for sub in range(2):
                            lt = lc * 4 + lsp * 2 + sub
                            nc.vector.tensor_add(
                                vext[lt][:, :, 0:DV],
                                ps[:, sub * 512:(sub + 1) * 512].rearrange(
                                    "p (h d) -> p h d", h=HPC),
                                vbias.rearrange("p (h d) -> p h d", h=HPC))
                            nc.vector.tensor_copy(vext[lt][:, :, DV], onesf)

            # ---- stage A2 generator: Q/K projection for pair m (64 mms) ----
            def a2_gen(m):
                first = [True]

                def tick():
                    if first[0]:
                        first[0] = False
                        return iter(())
                    return iter((None,))

                for w_tiles, xsrc, dst, btile in (
                    (wq_tiles, xtq, qt[m], qb_tile),
                    (wk_tiles, xtk, kt[m], kb_tile),
                ):
                    for lcp in range(2):
                        x_tiles = {}
                        for sub in range(2):
                            lc = lcp * 2 + sub
                            xa = xt_pool.tile([128, NI, 512], BF16, tag="xt",
                                              name="xp")
                            nc.sync.dma_start(
                                out=xa,
                                in_=xsrc[:, lc * 512:(lc + 1) * 512].rearrange(
                                    "(i p) c -> p i c", p=128))
                            x_tiles[sub] = [xa[:, i, :] for i in range(NI)]
                        ps = ap_pool.tile([128, QW], F32, tag="ap", name="psp")
                        for sub in range(2):
                            for i in range(NI):
                                yield from tick()
                                nc.tensor.matmul(
                                    ps[:, sub * 512:(sub + 1) * 512],
                                    lhsT=w_tiles[i][:, m * 128:(m + 1) * 128],
                                    rhs=x_tiles[sub][i],
                                    start=(i == 0), stop=(i == NI - 1))
                        for sub in range(2):
                            lc = lcp * 2 + sub
                            nc.vector.tensor_scalar(
                                out=dst[:, lc * 512:(lc + 1) * 512],
                                in0=ps[:, sub * 512:(sub + 1) * 512],
                                scalar1=btile[:, m:m + 1],
                                scalar2=None, op0=ADD)

            def drain(gen):
                if gen is None:
                    return
                for _ in gen:
                    pass

            # ---- pre-B: A2(0), then bulk V/C weights ----
            drain(a2_gen(0))
            nc.sync.dma_start(
                out=wv_all, in_=wvt[:, :].rearrange("(i p) c -> p i c", p=128))
            nc.sync.dma_start(
                out=wc_all, in_=wct[:, :].rearrange("(i p) c -> p i c", p=128))

            # ---- stage B ----
            # Units: (m, h2, qc); AV of unit u runs after scores of u+1.
            # Interleave: A1 into B(0) units 0-1 (4 mms/slot), A2(m+1) into
            # B(m) units 2-3 (2 mms/slot).
            units = []
            for m in range(NM):
                for h2 in range(2):
                    for qc in range(2):
                        units.append((m, h2, qc))

            exs = {}        # unit idx -> list of 16 ex tiles
            att_s = {}      # head -> staging tile
            pend_av = []    # queue of units whose AV is not yet emitted

            def emit_scores(ui, intl_gen, n_intl):
                m, h2, qc = units[ui]
                off = h2 * DV
                if m not in att_s:
                    att_s[m] = atts_pool.tile(
                        [128, NLT, 128], BF16, tag="atts", name=f"atts{m}")
                ex_list = []
                for k in range(NLT):
                    if intl_gen is not None:
                        for _ in range(n_intl):
                            if next(intl_gen, "END") == "END":
                                intl_gen = None
                                break
                    st = st_pool.tile([128, QW], F32, tag="st", name="st")
                    for j in range(2):
                        nc.tensor.matmul(
                            st[:, j * 512:(j + 1) * 512],
                            lhsT=kt[m][off:off + DV, k * 128:(k + 1) * 128],
                            rhs=qt[m][off:off + DV,
                                      qc * QW + j * 512:qc * QW + (j + 1) * 512],
                            start=True, stop=True)
                    ex = ex_pool.tile([128, QW], BF16, tag="ex", name="ex")
                    nc.scalar.activation(out=ex, in_=st, func=EXP, scale=0.125)
                    ex_list.append(ex)
                exs[ui] = ex_list
                return intl_gen

            def emit_av(ui):
                m, h2, qc = units[ui]
                h = m * 2 + h2
                ex_list = exs.pop(ui)
                ast = att_s[m]
                for qsg in range(2):
                    av = av_pool.tile([128, 4, DV + 1], F32, tag="av",
                                      name="av")
                    for qs in range(4):
                        qq = (qsg * 4 + qs) * 128
                        for k in range(NLT):
                            nc.tensor.matmul(
                                av[:, qs, :],
                                lhsT=ex_list[k][:, qq:qq + 128],
                                rhs=vext[k][:, h, :],
                                start=(k == 0), stop=(k == NLT - 1))
                    rc = rcp_pool.tile([128, 4], F32, tag="rc", name="rc")
                    nc.vector.reciprocal_approx_fast(out=rc, in_=av[:, :, DV])
                    for qs in range(4):
                        lt = qc * 8 + qsg * 4 + qs
                        nc.vector.tensor_scalar(
                            out=ast[:, lt, h2 * DV:(h2 + 1) * DV],
                            in0=av[:, qs, 0:DV],
                            scalar1=rc[:, qs:qs + 1],
                            scalar2=None, op0=MULT)
                if h2 == 1 and qc == 1:
                    # whole pair staged: xbar semantics
                    # out[fm, b, q] = in[q, b*128 + fm], fm = h2*64+dv
                    if m not in attnt:
                        attnt[m] = attnt_pool.tile([128, L], BF16, tag="attnt",
                                                   name=f"attnt{m}")
                    dst = attnt[m][:, :].rearrange("p (t q) -> p t q", t=NLT)
                    nc.sync.dma_start_transpose(dst, ast)
                    del att_s[m]

            attnt = {}
            if INTERLEAVE:
                intl = a1_gen()
            else:
                drain(a1_gen())
                for _m in range(1, NM):
                    drain(a2_gen(_m))
                intl = None
            n_per_slot = 4
            depth = 1 if PIPE else 0
            for ui in range(len(units)):
                m, h2, qc = units[ui]
                # refresh interleave source at unit boundaries
                if INTERLEAVE and ui % 4 == 2 and m < NM - 1:
                    drain(intl)
                    intl = a2_gen(m + 1)
                    n_per_slot = 2
                intl = emit_scores(ui, intl, n_per_slot)
                pend_av.append(ui)
                if len(pend_av) > depth:
                    emit_av(pend_av.pop(0))
            drain(intl)
            while pend_av:
                emit_av(pend_av.pop(0))

            # ---- stage C: output projection ----
            for lt in range(NLT):
                ps = st_pool.tile([128, QW], F32, tag="st", name="pso")
                for nck in range(2):
                    for dt in range(NM):
                        nc.tensor.matmul(
                            ps[:, nck * 512:(nck + 1) * 512],
                            lhsT=attnt[dt][:, lt * 128:(lt + 1) * 128],
                            rhs=wc_tiles[dt][:, nck * 512:(nck + 1) * 512],
                            start=(dt == 0), stop=(dt == NM - 1))
                ob = ob_pool.tile([128, QW], BF16, tag="ob", name="ob")
                if lt % 2 == 0:
                    nc.vector.tensor_copy(ob, ps)
                else:
                    nc.scalar.copy(ob, ps)
                nc.sync.dma_start(out=out[lt * 128:(lt + 1) * 128, :], in_=ob)

    nc.compile()
    return nc


def _get_nc():
    if "nc" not in _CACHE:
        _CACHE["nc"] = _build()
    return _CACHE["nc"]


def kernel(query, key, value, Wq, bq, Wk, bk, Wv, bv, Wc, bc, **_unused):
    query = np.asarray(query, np.float32)
    key = np.asarray(key, np.float32)
    value = np.asarray(value, np.float32)
    Wq = np.asarray(Wq, np.float32)
    Wk = np.asarray(Wk, np.float32)
    Wv = np.asarray(Wv, np.float32)
    Wc = np.asarray(Wc, np.float32)
    bq = np.asarray(bq, np.float32)
    bk = np.asarray(bk, np.float32)
    bv = np.asarray(bv, np.float32)
    bc = np.asarray(bc, np.float32)

    nc = _get_nc()

    def tb(x):
        return np.ascontiguousarray(x).astype(np_bf16)

    xtq = [tb(query[b].T) for b in range(B)]
    xtk = [tb(key[b].T) for b in range(B)]
    xtv = [tb(value[b].T) for b in range(B)]
    wqt_g = [tb(Wq[g * OC:(g + 1) * OC, :].T) for g in range(2)]
    wkt_g = [tb(Wk[g * OC:(g + 1) * OC, :].T) for g in range(2)]
    wvt_g = [tb(Wv[g * OC:(g + 1) * OC, :].T) for g in range(2)]
    wct_g = [tb(Wc[:, g * OC:(g + 1) * OC].T) for g in range(2)]

    in_maps = []
    for c in range(NCORES):
        b, g = c // 2, c % 2
        in_maps.append({
            "XTQ": xtq[b], "XTK": xtk[b], "XTV": xtv[b],
            "WQT": wqt_g[g], "WKT": wkt_g[g], "WVT": wvt_g[g],
            "WCT": wct_g[g],
            "BQ": np.ascontiguousarray(bq[g * OC:(g + 1) * OC]),
            "BK": np.ascontiguousarray(bk[g * OC:(g + 1) * OC]),
            "BV": np.ascontiguousarray(bv[g * OC:(g + 1) * OC]),
        })

    res = run_bass_kernel_spmd(nc, in_maps, core_ids=list(range(NCORES)),
                               **_CACHE.get("run_kwargs", {}))
    _CACHE["last_results"] = res

    outp = np.empty((B, L, D), np.float32)
    for b in range(B):
        outp[b] = (res.results[2 * b]["OUT"].astype(np.float32)
                   + res.results[2 * b + 1]["OUT"].astype(np.float32))
    outp += bc
    return outp


# revision 10
# speedup vs baseline: 1.1357x; 1.0066x over previous
"""Multi-head attention forward on 8 Trainium2 NeuronCores (Bass/Tile).

Problem: B=4, L=2048, D=1024, H=16 heads, DV=64.
  out = softmax((x_q Wq^T)(x_k Wk^T)^T / sqrt(DV)) (x_v Wv^T) Wc^T + biases

Sharding (8 cores): core c handles batch b = c//2 and head-group g = c%2
(8 heads = 512 of the 1024 projection columns). Each core produces a
full-shape [L, D] partial of the output projection; the host sums the two
partials per batch and adds bc.

All device data is bf16 (PSUM accumulation fp32); inputs are converted on
the host. Per-core pipeline:
  A1. V projection -> vext [128, 8, 65] bf16 per l-tile (ones column per
      head gives the softmax denominator through the AV matmul).
  A2. Q/K projections per head-pair -> qt/kt [128(2x64 dims), 2048] bf16.
  B.  Per (head, q-half) unit: 16 k-tiles of scores^T [k=128, q=1024] in
      PSUM -> ACT exp (scale=1/8) -> ex bf16; then AV flipped:
      out[q=128, 65] chains (lhsT=ex slice, rhs=vext) -> per-partition
      denominator -> reciprocal_approx_fast + tensor_scalar normalize
      into att_s [128, 64, 16]. Per head, one SBUF->SBUF DMA xbar
      transpose produces attnt [64, 2048] (= attn^T) for stage C.
      A1 matmuls are interleaved into B(0) score slots and A2(m+1)
      matmuls into B(m) slots so the PE fills the gaps of the ACT-bound
      exp stream.
  C.  Output projection out[l, 1024] accumulated over the 4 head pairs.
"""

import numpy as np

import concourse.bacc as bacc
import concourse.mybir as mybir
from concourse.tile import TileContext
from concourse.bass_utils import run_bass_kernel_spmd

try:
    from ml_dtypes import bfloat16 as np_bf16
except ImportError:  # pragma: no cover
    import jax.numpy as jnp
    np_bf16 = jnp.bfloat16

B, L, D, H = 4, 2048, 1024, 16
DV = 64
HPC = 8           # heads per core
OC = HPC * DV     # 512 projection cols per core
NCORES = 8

F32 = mybir.dt.float32
BF16 = mybir.dt.bfloat16
EXP = mybir.ActivationFunctionType.Exp
MULT = mybir.AluOpType.mult
ADD = mybir.AluOpType.add

NI = D // 128    # 8 contraction tiles for projections
NM = OC // 128   # 4 head pairs
NLT = L // 128   # 16 l/k tiles
QW = 1024        # q-half width in stage B

_CACHE = {}
PIPE = True
INTERLEAVE = True


def _build():
    nc = bacc.Bacc("TRN2", target_bir_lowering=False, debug=False,
                   num_devices=NCORES)

    xtq = nc.dram_tensor("XTQ", [D, L], BF16, kind="ExternalInput")
    xtk = nc.dram_tensor("XTK", [D, L], BF16, kind="ExternalInput")
    xtv = nc.dram_tensor("XTV", [D, L], BF16, kind="ExternalInput")
    wqt = nc.dram_tensor("WQT", [D, OC], BF16, kind="ExternalInput")
    wkt = nc.dram_tensor("WKT", [D, OC], BF16, kind="ExternalInput")
    wvt = nc.dram_tensor("WVT", [D, OC], BF16, kind="ExternalInput")
    wct = nc.dram_tensor("WCT", [OC, D], BF16, kind="ExternalInput")
    bqd = nc.dram_tensor("BQ", [OC], F32, kind="ExternalInput")
    bkd = nc.dram_tensor("BK", [OC], F32, kind="ExternalInput")
    bvd = nc.dram_tensor("BV", [OC], F32, kind="ExternalInput")
    out = nc.dram_tensor("OUT", [L, D], BF16, kind="ExternalOutput")

    with TileContext(nc) as tc:
        with (
            tc.tile_pool(name="const", bufs=1) as const_pool,
            tc.tile_pool(name="wcp", bufs=1) as wc_pool,
            tc.tile_pool(name="wvp", bufs=1) as wv_pool,
            tc.tile_pool(name="wqp", bufs=1) as wq_pool,
            tc.tile_pool(name="wkp", bufs=1) as wk_pool,
            tc.tile_pool(name="xt", bufs=4) as xt_pool,
            tc.tile_pool(name="qkt", bufs=2 * NM) as qkt_pool,
            tc.tile_pool(name="vext", bufs=NLT) as vext_pool,
            tc.tile_pool(name="ex", bufs=32) as ex_pool,
            tc.tile_pool(name="atts", bufs=2) as atts_pool,
            tc.tile_pool(name="attnt", bufs=NM) as attnt_pool,
            tc.tile_pool(name="rcp", bufs=4) as rcp_pool,
            tc.tile_pool(name="ob", bufs=2) as ob_pool,
            tc.tile_pool(name="st", bufs=2, space="PSUM") as st_pool,
            tc.tile_pool(name="ap", bufs=1, space="PSUM") as ap_pool,
            tc.tile_pool(name="av", bufs=2, space="PSUM") as av_pool,
        ):
            # ---- constants / weights ----
            qb_tile = const_pool.tile([128, NM], F32, tag="bq", name="bqt")
            nc.sync.dma_start(
                out=qb_tile, in_=bqd[:].rearrange("(m p) -> p m", p=128))
            kb_tile = const_pool.tile([128, NM], F32, tag="bk", name="bkt")
            nc.sync.dma_start(
                out=kb_tile, in_=bkd[:].rearrange("(m p) -> p m", p=128))
            vbias = const_pool.tile([128, OC], F32, tag="bv", name="bvt")
            nc.sync.dma_start(
                out=vbias, in_=bvd[:].unsqueeze(0).to_broadcast((128, OC)))
            onesf = const_pool.tile([128, HPC], F32, tag="ones", name="ones")
            nc.vector.memset(onesf, 1.0)

            wq_all = wq_pool.tile([128, NI, OC], BF16, tag="wq", name="wq_a")
            nc.sync.dma_start(
                out=wq_all, in_=wqt[:, :].rearrange("(i p) c -> p i c", p=128))
            wk_all = wk_pool.tile([128, NI, OC], BF16, tag="wk", name="wk_a")
            nc.sync.dma_start(
                out=wk_all, in_=wkt[:, :].rearrange("(i p) c -> p i c", p=128))
            wq_tiles = [wq_all[:, i, :] for i in range(NI)]
            wk_tiles = [wk_all[:, i, :] for i in range(NI)]

            wv_all = wv_pool.tile([128, NI, OC], BF16, tag="wv", name="wv_a")
            wv_tiles = [wv_all[:, i, :] for i in range(NI)]
            wc_all = wc_pool.tile([128, NM, D], BF16, tag="wc", name="wc_a")
            wc_tiles = [wc_all[:, i, :] for i in range(NM)]

            qt = [qkt_pool.tile([128, L], BF16, tag="qkt", name=f"qt{i}")
                  for i in range(NM)]
            kt = [qkt_pool.tile([128, L], BF16, tag="qkt", name=f"kt{i}")
                  for i in range(NM)]
            vext = [vext_pool.tile([128, HPC, DV + 1], BF16, tag="vext",
                                   name=f"vext{i}")
                    for i in range(NLT)]

            # ---- stage A1 generator: V projection (128 mms + DVE) ----
            # Yields BEFORE each matmul (except the first) so the pull that
            # consumes the last matmul also emits the trailing DVE writes.
            def a1_gen():
                first = [True]

                def tick():
                    if first[0]:
                        first[0] = False
                        return iter(())
                    return iter((None,))

                xas = {}

                def load_v(lc):
                    xa = xt_pool.tile([128, NI, 512], BF16, tag="xt",
                                      name="xv")
                    nc.sync.dma_start(
                        out=xa,
                        in_=xtv[:, lc * 512:(lc + 1) * 512].rearrange(
                            "(i p) c -> p i c", p=128))
                    xas[lc] = xa

                load_v(0)
                load_v(1)
                for lc in range(4):
                    if lc + 2 < 4:
                        load_v(lc + 2)
                    xa = xas.pop(lc)
                    x_tiles = [xa[:, i, :] for i in range(NI)]
                    for lsp in range(2):
                        ps = ap_pool.tile([128, QW], F32, tag="ap", name="psv")
                        for sub in range(2):
                            ls = lsp * 2 + sub
                            for i in range(NI):
                                yield from tick()
                                nc.tensor.matmul(
                                    ps[:, sub * 512:(sub + 1) * 512],
                                    lhsT=x_tiles[i][:, ls * 128:(ls + 1) * 128],
                                    rhs=wv_tiles[i],
                                    start=(i == 0), stop=(i == NI - 1))
                        for sub in range(2):
                            lt = lc * 4 + lsp * 2 + sub
                            nc.vector.tensor_add(
                                vext[lt][:, :, 0:DV],
                                ps[:, sub * 512:(sub + 1) * 512].rearrange(
                                    "p (h d) -> p h d", h=HPC),
                                vbias.rearrange("p (h d) -> p h d", h=HPC))
                            nc.vector.tensor_copy(vext[lt][:, :, DV], onesf)

            # ---- stage A2 generator: Q/K projection for pair m (64 mms) ----
            def a2_gen(m):
                first = [True]

                def tick():
                    if first[0]:
                        first[0] = False
                        return iter(())
                    return iter((None,))

                xas = {}

                def load_x(xsrc, lcp):
                    for sub in range(2):
                        lc = lcp * 2 + sub
                        xa = xt_pool.tile([128, NI, 512], BF16, tag="xt",
                                          name="xp")
                        nc.sync.dma_start(
                            out=xa,
                            in_=xsrc[:, lc * 512:(lc + 1) * 512].rearrange(
                                "(i p) c -> p i c", p=128))
                        xas[(id(xsrc), lcp, sub)] = xa

                srcs = ((wq_tiles, xtq, qt[m], qb_tile),
                        (wk_tiles, xtk, kt[m], kb_tile))
                load_x(srcs[0][1], 0)
                for si, (w_tiles, xsrc, dst, btile) in enumerate(srcs):
                    for lcp in range(2):
                        # prefetch next chunk
                        if lcp == 0:
                            load_x(xsrc, 1)
                        elif si == 0:
                            load_x(srcs[1][1], 0)
                        x_tiles = {
                            sub: [xas[(id(xsrc), lcp, sub)][:, i, :]
                                  for i in range(NI)]
                            for sub in range(2)
                        }
                        for sub in range(2):
                            xas.pop((id(xsrc), lcp, sub))
                        ps = ap_pool.tile([128, QW], F32, tag="ap", name="psp")
                        for sub in range(2):
                            for i in range(NI):
                                yield from tick()
                                nc.tensor.matmul(
                                    ps[:, sub * 512:(sub + 1) * 512],
                                    lhsT=w_tiles[i][:, m * 128:(m + 1) * 128],
                                    rhs=x_tiles[sub][i],
                                    start=(i == 0), stop=(i == NI - 1))
                        for sub in range(2):
                            lc = lcp * 2 + sub
                            nc.vector.tensor_scalar(
                                out=dst[:, lc * 512:(lc + 1) * 512],
                                in0=ps[:, sub * 512:(sub + 1) * 512],
                                scalar1=btile[:, m:m + 1],
                                scalar2=None, op0=ADD)

            def drain(gen):
                if gen is None:
                    return
                for _ in gen:
                    pass

            # ---- pre-B: A2(0), then bulk V/C weights ----
            drain(a2_gen(0))
            nc.sync.dma_start(
                out=wv_all, in_=wvt[:, :].rearrange("(i p) c -> p i c", p=128))
            nc.sync.dma_start(
                out=wc_all, in_=wct[:, :].rearrange("(i p) c -> p i c", p=128))

            # ---- stage B ----
            # Units: (m, h2, qc); AV of unit u runs after scores of u+1.
            # Interleave: A1 into B(0) units 0-1 (4 mms/slot), A2(m+1) into
            # B(m) units 2-3 (2 mms/slot).
            units = []
            for m in range(NM):
                for h2 in range(2):
                    for qc in range(2):
                        units.append((m, h2, qc))

            exs = {}        # unit idx -> list of 16 ex tiles
            att_s = {}      # head -> staging tile
            pend_av = []    # queue of units whose AV is not yet emitted

            exs = {}        # unit idx -> list of 16 ex tiles
            att_s = {}      # pair -> staging tile
            avt = {}        # (ui, qsg) -> av psum tile
            attnt = {}

            def emit_chain(pu, j):
                m, h2, qc = units[pu]
                h = m * 2 + h2
                qsg, qs = j // 4, j % 4
                if (pu, qsg) not in avt:
                    avt[(pu, qsg)] = av_pool.tile(
                        [128, 4, DV + 1], F32, tag="av", name="av")
                av = avt[(pu, qsg)]
                ex_list = exs[pu]
                qq = (qsg * 4 + qs) * 128
                for k in range(NLT):
                    nc.tensor.matmul(
                        av[:, qs, :],
                        lhsT=ex_list[k][:, qq:qq + 128],
                        rhs=vext[k][:, h, :],
                        start=(k == 0), stop=(k == NLT - 1))

            def emit_norm(pu, qsg):
                m, h2, qc = units[pu]
                av = avt.pop((pu, qsg))
                ast = att_s[m]
                rc = rcp_pool.tile([128, 4], F32, tag="rc", name="rc")
                nc.vector.reciprocal_approx_fast(out=rc, in_=av[:, :, DV])
                for qs in range(4):
                    lt = qc * 8 + qsg * 4 + qs
                    nc.vector.tensor_scalar(
                        out=ast[:, lt, h2 * DV:(h2 + 1) * DV],
                        in0=av[:, qs, 0:DV],
                        scalar1=rc[:, qs:qs + 1],
                        scalar2=None, op0=MULT)
                if qsg == 1:
                    exs.pop(pu)
                    if h2 == 1 and qc == 1:
                        # pair complete: xbar transpose
                        # out[fm, b, q] = in[q, b*128 + fm], fm = h2*64+dv
                        attnt[m] = attnt_pool.tile(
                            [128, L], BF16, tag="attnt", name=f"attnt{m}")
                        dst = attnt[m][:, :].rearrange(
                            "p (t q) -> p t q", t=NLT)
                        nc.sync.dma_start_transpose(dst, ast)
                        del att_s[m]

            # prev-unit AV action schedules: k-slot -> actions
            # c<j> = chain j, n<g> = normalize qsg g ("end" after slot 15)
            SPREAD = {2: ["c0"], 4: ["c1"], 6: ["c2"], 8: ["c3"], 9: ["c4"],
                      10: ["n0"], 11: ["c5"], 13: ["c6"], 15: ["c7"],
                      "end": ["n1"]}
            SQUEEZE = {11: ["c0", "c1"], 12: ["c2", "c3"],
                       13: ["c4", "c5", "n0"], 14: ["c6", "c7"],
                       "end": ["n1"]}

            def do_actions(prev, acts):
                for a in acts:
                    if a[0] == "c":
                        emit_chain(prev, int(a[1]))
                    else:
                        emit_norm(prev, int(a[1]))

            def emit_unit(ui, intl_gen, n_intl, prev, sched):
                m, h2, qc = units[ui]
                off = h2 * DV
                if m not in att_s:
                    att_s[m] = atts_pool.tile(
                        [128, NLT, 128], BF16, tag="atts", name=f"atts{m}")
                ex_list = []
                for k in range(NLT):
                    if intl_gen is not None:
                        for _ in range(n_intl):
                            if next(intl_gen, "END") == "END":
                                intl_gen = None
                                break
                    if prev is not None:
                        do_actions(prev, sched.get(k, ()))
                    st = st_pool.tile([128, QW], F32, tag="st", name="st")
                    for j in range(2):
                        nc.tensor.matmul(
                            st[:, j * 512:(j + 1) * 512],
                            lhsT=kt[m][off:off + DV, k * 128:(k + 1) * 128],
                            rhs=qt[m][off:off + DV,
                                      qc * QW + j * 512:qc * QW + (j + 1) * 512],
                            start=True, stop=True)
                    ex = ex_pool.tile([128, QW], BF16, tag="ex", name="ex")
                    nc.scalar.activation(out=ex, in_=st, func=EXP, scale=0.125)
                    ex_list.append(ex)
                exs[ui] = ex_list
                if prev is not None:
                    do_actions(prev, sched.get("end", ()))
                return intl_gen

            if INTERLEAVE:
                intl = a1_gen()
            else:
                drain(a1_gen())
                for _m in range(1, NM):
                    drain(a2_gen(_m))
                intl = None
            n_per_slot = 5
            for ui in range(len(units)):
                m, h2, qc = units[ui]
                if INTERLEAVE and ui % 4 == 2 and m < NM - 1:
                    drain(intl)
                    intl = a2_gen(m + 1)
                    n_per_slot = 2
                sched = SQUEEZE if ui == 1 else SPREAD
                intl = emit_unit(ui, intl, n_per_slot,
                                 ui - 1 if ui > 0 else None, sched)
            drain(intl)
            # drain last unit's AV
            last = len(units) - 1
            for j in range(8):
                emit_chain(last, j)
                if j == 3:
                    emit_norm(last, 0)
            emit_norm(last, 1)

            # ---- stage C: output projection ----
            for lt in range(NLT):
                ps = st_pool.tile([128, QW], F32, tag="st", name="pso")
                for nck in range(2):
                    for dt in range(NM):
                        nc.tensor.matmul(
                            ps[:, nck * 512:(nck + 1) * 512],
                            lhsT=attnt[dt][:, lt * 128:(lt + 1) * 128],
                            rhs=wc_tiles[dt][:, nck * 512:(nck + 1) * 512],
                            start=(dt == 0), stop=(dt == NM - 1))
                ob = ob_pool.tile([128, QW], BF16, tag="ob", name="ob")
                if lt % 2 == 0:
                    nc.vector.tensor_copy(ob, ps)
                else:
                    nc.scalar.copy(ob, ps)
                nc.sync.dma_start(out=out[lt * 128:(lt + 1) * 128, :], in_=ob)

    nc.compile()
    return nc


def _get_nc():
    if "nc" not in _CACHE:
        _CACHE["nc"] = _build()
    return _CACHE["nc"]


def kernel(query, key, value, Wq, bq, Wk, bk, Wv, bv, Wc, bc, **_unused):
    query = np.asarray(query, np.float32)
    key = np.asarray(key, np.float32)
    value = np.asarray(value, np.float32)
    Wq = np.asarray(Wq, np.float32)
    Wk = np.asarray(Wk, np.float32)
    Wv = np.asarray(Wv, np.float32)
    Wc = np.asarray(Wc, np.float32)
    bq = np.asarray(bq, np.float32)
    bk = np.asarray(bk, np.float32)
    bv = np.asarray(bv, np.float32)
    bc = np.asarray(bc, np.float32)

    nc = _get_nc()

    def tb(x):
        return np.ascontiguousarray(x).astype(np_bf16)

    xtq = [tb(query[b].T) for b in range(B)]
    xtk = [tb(key[b].T) for b in range(B)]
    xtv = [tb(value[b].T) for b in range(B)]
    wqt_g = [tb(Wq[g * OC:(g + 1) * OC, :].T) for g in range(2)]
    wkt_g = [tb(Wk[g * OC:(g + 1) * OC, :].T) for g in range(2)]
    wvt_g = [tb(Wv[g * OC:(g + 1) * OC, :].T) for g in range(2)]
    wct_g = [tb(Wc[:, g * OC:(g + 1) * OC].T) for g in range(2)]

    in_maps = []
    for c in range(NCORES):
        b, g = c // 2, c % 2
        in_maps.append({
            "XTQ": xtq[b], "XTK": xtk[b], "XTV": xtv[b],
            "WQT": wqt_g[g], "WKT": wkt_g[g], "WVT": wvt_g[g],
            "WCT": wct_g[g],
            "BQ": np.ascontiguousarray(bq[g * OC:(g + 1) * OC]),
            "BK": np.ascontiguousarray(bk[g * OC:(g + 1) * OC]),
            "BV": np.ascontiguousarray(bv[g * OC:(g + 1) * OC]),
        })

    res = run_bass_kernel_spmd(nc, in_maps, core_ids=list(range(NCORES)),
                               **_CACHE.get("run_kwargs", {}))
    _CACHE["last_results"] = res

    outp = np.empty((B, L, D), np.float32)
    for b in range(B):
        outp[b] = (res.results[2 * b]["OUT"].astype(np.float32)
                   + res.results[2 * b + 1]["OUT"].astype(np.float32))
    outp += bc
    return outp
